# revision 10
# baseline (speedup 1.0000x reference)
"""Causal self-attention (B=4, T=2048, C=1024, H=16) on 8 Trainium2 cores.

Sharding: core c -> batch b = c//2, heads 8*(c%2) .. 8*(c%2)+7.
Each core computes the QKV projection for its 8 heads, causal attention, and a
row-sharded c_proj partial [2048, 1024]. Host sums the two partials per batch
and adds b_proj.

All on-chip tensors are kept transposed (contraction dim on partitions) so no
transposes are needed anywhere:
  - x^T [C, T] streamed from HBM in 512-token chunks (re-read per head pair)
  - per head pair: Q^T/K^T tiles [128, T]; partitions 0:64 = even head dims,
    64:128 = odd head dims (matmul operands must share a base partition)
  - S^T[k, q] = sum_d K[k,d] Q[q,d] via matmul(lhsT=K^T block, rhs=Q^T chunk)
  - E^T = exp(S^T/8)  (no max subtraction needed: scores are ~N(0,1))
  - causal mask applied only on diagonal tiles via a sliding mask strip
  - [O^T; denom] = [V | 1]^T @ E^T accumulated over key blocks (ones column
    appended to V gives the softmax denominator for free)
  - normalize: R = ones x (1/denom) via a K=1 matmul, Y^T = O^T * R
  - out = sum_a Y^T_a.T @ Wp_a  (row-sharded c_proj partial)
"""

import numpy as np

import concourse.bass as bass
import concourse.bacc as bacc
import concourse.mybir as mybir
import concourse.tile as tile
from concourse.bass_utils import run_bass_kernel_spmd

# problem constants (hardcoded per contract)
B, T, C, H = 4, 2048, 1024, 16
HD = 64            # head dim
NH = 8             # heads per core
NPAIR = NH // 2    # head pairs per core
DV = NH * HD       # 512 v cols per core
N_CORES = 8
P = 128            # partitions
TQ = 512           # token chunk / psum free size
NKT = C // P       # 8 contraction tiles for the projections
NTT = T // P       # 16 token tiles
NCH = T // TQ      # 4 token chunks

F32 = mybir.dt.float32
F32R = mybir.dt.float32r
EXP = mybir.ActivationFunctionType.Exp

# matmul input interpretation: "f32r" = full-rate reduced-precision fp32,
# "f32" = exact fp32 at quarter rate.
MM_DTYPE = "f32r"


# dtype for every tensor that feeds the PE as a matmul operand; walrus
# requires fp32r matmul inputs to be produced (rounded) as fp32r, so the
# tiles/DRAM tensors are declared fp32r directly (same bits as fp32 on host).
MMDT = F32R if MM_DTYPE == "f32r" else F32


def _mm(ap):
    return ap


def _build_program(has_bqk: bool, has_bv: bool, repeat: int = 1):
    nc = bacc.Bacc(None, target_bir_lowering=False)

    xT_d = nc.dram_tensor("xT", [C, T], MMDT, kind="ExternalInput")
    wqk_d = nc.dram_tensor("wqk", [C, 4 * P * 2], MMDT, kind="ExternalInput")
    wv_d = nc.dram_tensor("wv", [C, DV], MMDT, kind="ExternalInput")
    wp_d = nc.dram_tensor("wp", [DV, C], MMDT, kind="ExternalInput")
    mask_d = nc.dram_tensor("mask", [P, 896], MMDT, kind="ExternalInput")
    ones_d = nc.dram_tensor("ones", [P, TQ], MMDT, kind="ExternalInput")
    if has_bqk:
        bqk_d = nc.dram_tensor("bqk", [1, 4 * P * 2], MMDT, kind="ExternalInput")
    if has_bv:
        bv_d = nc.dram_tensor("bv", [1, DV], MMDT, kind="ExternalInput")
    out_d = nc.dram_tensor("out", [T, C], F32, kind="ExternalOutput")

    xT_r = xT_d[:].rearrange("(k p) t -> p k t", p=P)    # [P, NKT, T]
    wqk_r = wqk_d[:].rearrange("(k p) m -> p k m", p=P)  # [P, NKT, 1024]

    with nc.allow_low_precision("fp32r matmul path"), tile.TileContext(nc) as tc:
        with (
            tc.tile_pool(name="xc", bufs=2) as xc_pool,
            tc.tile_pool(name="qk2", bufs=2) as qk2_pool,
            tc.tile_pool(name="wqk", bufs=2) as wqk_pool,
            tc.tile_pool(name="wconst", bufs=1) as wconst_pool,
            tc.tile_pool(name="vp", bufs=NTT) as vp_pool,
            tc.tile_pool(name="yt", bufs=NPAIR) as yt_pool,
            tc.tile_pool(name="small", bufs=1) as small_pool,
            tc.tile_pool(name="et", bufs=3) as et_pool,
            tc.tile_pool(name="ot", bufs=3) as ot_pool,
            tc.tile_pool(name="osb", bufs=2) as osb_pool,
            tc.tile_pool(name="recip", bufs=2) as recip_pool,
            tc.tile_pool(name="psmm", bufs=3, space="PSUM") as psmm_pool,
            tc.tile_pool(name="pso", bufs=2, space="PSUM") as pso_pool,
            tc.tile_pool(name="psr", bufs=1, space="PSUM") as psr_pool,
        ):
            # ---- resident loads ----
            wv_sb = wconst_pool.tile([P, NKT, DV], MMDT, name="wv_sb")
            nc.sync.dma_start(wv_sb[:], wv_d[:].rearrange("(k p) n -> p k n", p=P))

            wp_sb = wconst_pool.tile([P, DV // P, C], MMDT, name="wp_sb")
            nc.sync.dma_start(wp_sb[:], wp_d[:].rearrange("(a p) n -> p a n", p=P))

            mask_sb = small_pool.tile([P, 896], MMDT, name="mask_sb")
            nc.sync.dma_start(mask_sb[:], mask_d[:])

            ones_sb = small_pool.tile([P, TQ], MMDT, name="ones_sb")
            nc.sync.dma_start(ones_sb[:], ones_d[:])

            if has_bqk:
                bqk_sb = small_pool.tile([1, 4 * P * 2], MMDT, name="bqk_sb")
                nc.sync.dma_start(bqk_sb[:], bqk_d[:])
            if has_bv:
                bv_sb = small_pool.tile([1, DV], MMDT, name="bv_sb")
                nc.sync.dma_start(bv_sb[:], bv_d[:])

            import contextlib
            rep_ctx = tc.For_i(0, repeat, 1) if repeat > 1 else contextlib.nullcontext()
            with rep_ctx:
                _run_body(
                    nc, tc, has_bqk, has_bv,
                    xT_r, wqk_r, out_d,
                    wv_sb, wp_sb, mask_sb, ones_sb,
                    bqk_sb if has_bqk else None, bv_sb if has_bv else None,
                    xc_pool, qk2_pool, wqk_pool, vp_pool, yt_pool,
                    et_pool, ot_pool, osb_pool, recip_pool,
                    psmm_pool, pso_pool, psr_pool,
                )

    nc.compile()
    return nc


def _run_body(nc, tc, has_bqk, has_bv, xT_r, wqk_r, out_d,
              wv_sb, wp_sb, mask_sb, ones_sb, bqk_sb, bv_sb,
              xc_pool, qk2_pool, wqk_pool, vp_pool, yt_pool,
              et_pool, ot_pool, osb_pool, recip_pool,
              psmm_pool, pso_pool, psr_pool):
            vtiles = [None] * NTT
            ytiles = [
                yt_pool.tile([P, T], MMDT, name=f"yt{a}", tag="yt")
                for a in range(NPAIR)
            ]

            for pair in range(NPAIR):
                # ---- production: Q^T/K^T for this pair (+ V' during pair 0) ----
                wqk_h = wqk_pool.tile([P, NKT, 2 * P], MMDT, name=f"wqkh{pair}", tag="wqk")
                nc.sync.dma_start(wqk_h[:], wqk_r[:, :, 2 * P * pair:2 * P * (pair + 1)])

                qk2 = qk2_pool.tile([P, 2, T], MMDT, name=f"qk2_{pair}", tag="qk2")

                for t in range(NCH):
                    xct = xc_pool.tile([P, NKT, TQ], MMDT, name=f"xct{pair}_{t}", tag="xc")
                    nc.sync.dma_start(xct[:], xT_r[:, :, t * TQ:(t + 1) * TQ])

                    if pair == 0:
                        # V' tiles [P tokens, NH, HD+1] (ones col appended)
                        for loc in range(TQ // P):
                            tt = t * (TQ // P) + loc
                            ps = psmm_pool.tile([P, DV], F32, name=f"psv{tt}", tag="mm")
                            for k in range(NKT):
                                nc.tensor.matmul(
                                    ps[:],
                                    _mm(xct[:, k, loc * P:(loc + 1) * P]),
                                    _mm(wv_sb[:, k, :]),
                                    start=(k == 0),
                                    stop=(k == NKT - 1 and not has_bv),
                                )
                            if has_bv:
                                nc.tensor.matmul(
                                    ps[:], _mm(ones_sb[0:1, 0:P]), _mm(bv_sb[:]),
                                    start=False, stop=True,
                                )
                            vt = vp_pool.tile([P, NH, HD + 1], MMDT, name=f"vt{tt}", tag="vt")
                            nc.vector.tensor_copy(
                                vt[:, :, 0:HD], ps[:].rearrange("p (h d) -> p h d", d=HD)
                            )
                            nc.vector.tensor_copy(
                                vt[:, :, HD:HD + 1],
                                ones_sb[:, 0:NH].rearrange("p (h o) -> p h o", o=1),
                            )
                            vtiles[tt] = vt

                    for qk in range(2):  # 0 -> Q^T, 1 -> K^T
                        ps = psmm_pool.tile([P, TQ], F32, name=f"psqk{pair}_{t}_{qk}", tag="mm")
                        for k in range(NKT):
                            nc.tensor.matmul(
                                ps[:],
                                _mm(wqk_h[:, k, qk * P:(qk + 1) * P]),
                                _mm(xct[:, k, :]),
                                start=(k == 0),
                                stop=(k == NKT - 1 and not has_bqk),
                            )
                        if has_bqk:
                            nc.tensor.matmul(
                                ps[:],
                                _mm(bqk_sb[:, 2 * P * pair + qk * P:2 * P * pair + (qk + 1) * P]),
                                _mm(ones_sb[0:1, :]),
                                start=False, stop=True,
                            )
                        nc.vector.tensor_copy(qk2[:, qk, t * TQ:(t + 1) * TQ], ps[:])

                # ---- attention for the two heads of this pair ----
                for j in range(2):
                    h = 2 * pair + j
                    qT = qk2[64 * j:64 * j + HD, 0, :]
                    kT = qk2[64 * j:64 * j + HD, 1, :]
                    for qg in range(NCH):
                        nkb = (qg + 1) * (TQ // P)
                        ps_o = pso_pool.tile([HD + 1, TQ], F32, name=f"pso{h}_{qg}", tag="o")
                        for kb in range(nkb):
                            ps_s = psmm_pool.tile([P, TQ], F32, name=f"pss{h}_{qg}_{kb}", tag="mm")
                            nc.tensor.matmul(
                                ps_s[:],
                                _mm(kT[:, kb * P:(kb + 1) * P]),
                                _mm(qT[:, qg * TQ:(qg + 1) * TQ]),
                                start=True, stop=True,
                            )
                            et = et_pool.tile([P, TQ], MMDT, name=f"et{h}_{qg}_{kb}", tag="et")
                            # E^T = exp(S^T / sqrt(hd))
                            nc.scalar.activation(et[:], ps_s[:], EXP, scale=1.0 / np.sqrt(HD))
                            r = kb - qg * (TQ // P)
                            if r >= 0:
                                # diagonal tile: zero the strictly-upper (k > q) part
                                nc.vector.tensor_mul(
                                    et[:], et[:],
                                    mask_sb[:, 384 - 128 * r:384 - 128 * r + TQ],
                                )
                            nc.tensor.matmul(
                                ps_o[:],
                                _mm(vtiles[kb][:, h, :]),
                                _mm(et[:]),
                                start=(kb == 0),
                                stop=(kb == nkb - 1),
                            )
                        # normalize: Y^T = O^T * (ones x 1/denom)
                        recip = recip_pool.tile([1, TQ], MMDT, name=f"recip{h}_{qg}", tag="recip")
                        nc.vector.reciprocal(recip[:], ps_o[HD:HD + 1, :])
                        ps_r = psr_pool.tile([HD, TQ], F32, name=f"psr{h}_{qg}", tag="r")
                        nc.tensor.matmul(
                            ps_r[:], _mm(ones_sb[0:1, 0:HD]), _mm(recip[:]),
                            start=True, stop=True,
                        )
                        o_sb = osb_pool.tile([HD, TQ], F32, name=f"osb{h}_{qg}", tag="osb")
                        nc.scalar.copy(o_sb[:], ps_o[0:HD, :])
                        yt_slice = ytiles[pair][
                            64 * j:64 * j + HD, qg * TQ:(qg + 1) * TQ
                        ]
                        nc.vector.tensor_mul(yt_slice, o_sb[:], ps_r[:])

            # ---- c_proj partial: out[t, c] = sum_a Y^T[a, t] * Wp[a, c] ----
            for tt in range(NTT):
                for nn in range(C // TQ):
                    ps = psmm_pool.tile([P, TQ], F32, name=f"pspr{tt}_{nn}", tag="mm")
                    for a in range(DV // P):
                        nc.tensor.matmul(
                            ps[:],
                            _mm(ytiles[a][:, tt * P:(tt + 1) * P]),
                            _mm(wp_sb[:, a, nn * TQ:(nn + 1) * TQ]),
                            start=(a == 0),
                            stop=(a == DV // P - 1),
                        )
                    ot = ot_pool.tile([P, TQ], F32, name=f"ot{tt}_{nn}", tag="ot")
                    nc.scalar.copy(ot[:], ps[:])
                    nc.sync.dma_start(
                        out_d[tt * P:(tt + 1) * P, nn * TQ:(nn + 1) * TQ], ot[:]
                    )


_PROG_CACHE = {}


def _get_program(has_bqk: bool, has_bv: bool, repeat: int = 1):
    key = (has_bqk, has_bv, MM_DTYPE, repeat)
    if key not in _PROG_CACHE:
        _PROG_CACHE[key] = _build_program(has_bqk, has_bv, repeat)
    return _PROG_CACHE[key]


def make_mask() -> np.ndarray:
    # mask[i, c] = 1 iff i <= c - 384; sliced at offset 384-128r this gives
    # valid (k<=q) iff 128r + i <= j for the r-th diagonal tile.
    i = np.arange(P)[:, None]
    c = np.arange(896)[None, :]
    return (i <= c - 384).astype(np.float32)


def shard_inputs(x, w_attn, b_attn, w_proj):
    """Build the 8 per-core input maps."""
    mask = make_mask()
    has_bqk = bool(np.any(b_attn[:2 * C]))
    has_bv = bool(np.any(b_attn[2 * C:]))
    in_maps = []
    for core in range(N_CORES):
        b, half = divmod(core, 2)
        hs = half * 512  # col offset into the q/k/v regions
        # per-pair blocks of 256 cols: [Q_2i | Q_2i+1 | K_2i | K_2i+1]
        wq = w_attn[:, hs:hs + 512].reshape(C, NPAIR, 2 * HD)
        wk = w_attn[:, C + hs:C + hs + 512].reshape(C, NPAIR, 2 * HD)
        wqk = np.concatenate([wq, wk], axis=2).reshape(C, 4 * P * 2)
        m = {
            "xT": np.ascontiguousarray(np.asarray(x[b]).T),
            "wqk": np.ascontiguousarray(wqk),
            "wv": np.ascontiguousarray(w_attn[:, 2 * C + hs:2 * C + hs + 512]),
            "wp": np.ascontiguousarray(w_proj[hs:hs + 512, :]),
            "mask": mask,
            "ones": np.ones((P, TQ), dtype=np.float32),
        }
        if has_bqk:
            bq = b_attn[hs:hs + 512].reshape(NPAIR, 2 * HD)
            bk = b_attn[C + hs:C + hs + 512].reshape(NPAIR, 2 * HD)
            m["bqk"] = np.ascontiguousarray(
                np.concatenate([bq, bk], axis=1).reshape(1, 4 * P * 2)
            ).astype(np.float32)
        if has_bv:
            m["bv"] = np.ascontiguousarray(
                b_attn[2 * C + hs:2 * C + hs + 512].reshape(1, DV)
            ).astype(np.float32)
        in_maps.append(m)
    return in_maps, has_bqk, has_bv


def kernel(x, w_attn, b_attn, w_proj, b_proj):
    x = np.asarray(x, dtype=np.float32)
    w_attn = np.asarray(w_attn, dtype=np.float32)
    b_attn = np.asarray(b_attn, dtype=np.float32)
    w_proj = np.asarray(w_proj, dtype=np.float32)
    b_proj = np.asarray(b_proj, dtype=np.float32)

    in_maps, has_bqk, has_bv = shard_inputs(x, w_attn, b_attn, w_proj)
    nc = _get_program(has_bqk, has_bv)
    res = run_bass_kernel_spmd(nc, in_maps, list(range(N_CORES))).results

    out = np.empty((B, T, C), dtype=np.float32)
    for b in range(B):
        out[b] = res[2 * b]["out"] + res[2 * b + 1]["out"] + b_proj
    return out


# revision 27
# speedup vs baseline: 377.8116x; 377.8116x over previous
"""Causal self-attention (B=4, T=2048, C=1024, H=16) on 8 Trainium2 cores.

Sharding: core c -> batch b = c//2, heads 8*(c%2) .. 8*(c%2)+7.
Each core computes the QKV projection for its 8 heads, causal attention, and a
row-sharded c_proj partial [2048, 1024]. Host sums the two partials per batch
and adds b_proj.

All on-chip tensors are kept transposed (contraction dim on partitions) so no
transposes are needed anywhere:
  - x^T [C, T] streamed from HBM in 512-token chunks (re-read per head pair)
  - per head pair: Q^T/K^T tile [128, 2, T]; partitions 0:64 = even head dims,
    64:128 = odd head dims (matmul operands must share a base partition)
  - S^T[k, q] = sum_d K[k,d] Q[q,d] via matmul(lhsT=K^T block, rhs=Q^T chunk)
  - E^T = exp(S^T/8)  (no max subtraction needed: scores are ~N(0,1));
    on diagonal tiles only the valid column range is exp'd, the masked prefix
    is zero-filled from the mask strip's all-zero region, and a 128-wide
    diagonal band is multiplied by the triangular mask
  - [O^T; denom] = [V | 1]^T @ E^T accumulated over key blocks (ones column
    appended to V gives the softmax denominator for free)
  - normalize: R = ones x (1/denom) via a K=1 matmul, Y^T = O^T * R
  - out = sum_a Y^T_a.T @ Wp_a  (row-sharded c_proj partial)

Matmuls run as float32r (full-rate reduced-precision fp32): every tensor that
feeds the PE is declared fp32r (same bits as fp32 on host); walrus requires
fp32r matmul inputs to be produced as fp32r.
"""

import contextlib

import numpy as np

import concourse.bass as bass
import concourse.bacc as bacc
import concourse.mybir as mybir
import concourse.tile as tile
from concourse.bass_utils import run_bass_kernel_spmd

# problem constants (hardcoded per contract)
B, T, C, H = 4, 2048, 1024, 16
HD = 64            # head dim
NH = 8             # heads per core
NPAIR = NH // 2    # head pairs per core
DV = NH * HD       # 512 v cols per core
N_CORES = 8
P = 128            # partitions
TQ = 512           # token chunk / psum free size
NKT = C // P       # 8 contraction tiles for the projections
NTT = T // P       # 16 token tiles
NCH = T // TQ      # 4 token chunks

F32 = mybir.dt.float32
F32R = mybir.dt.float32r
EXP = mybir.ActivationFunctionType.Exp

# "f32r" = full-rate reduced-precision fp32 matmuls, "f32" = exact quarter-rate
MM_DTYPE = "f32r"
MMDT = F32R if MM_DTYPE == "f32r" else F32


def _build_program(has_bqk: bool, has_bv: bool, repeat: int = 1):
    nc = bacc.Bacc(None, target_bir_lowering=False)

    xT_d = nc.dram_tensor("xT", [C, T], MMDT, kind="ExternalInput")
    wqk_d = nc.dram_tensor("wqk", [C, 4 * P * 2], MMDT, kind="ExternalInput")
    wv_d = nc.dram_tensor("wv", [C, DV], MMDT, kind="ExternalInput")
    wp_d = nc.dram_tensor("wp", [DV, C], MMDT, kind="ExternalInput")
    mask_d = nc.dram_tensor("mask", [P, 512], MMDT, kind="ExternalInput")
    ones_d = nc.dram_tensor("ones", [P, TQ], MMDT, kind="ExternalInput")
    bqk_d = bv_d = None
    if has_bqk:
        bqk_d = nc.dram_tensor("bqk", [1, 4 * P * 2], MMDT, kind="ExternalInput")
    if has_bv:
        bv_d = nc.dram_tensor("bv", [1, DV], MMDT, kind="ExternalInput")
    out_d = nc.dram_tensor("out", [T, C], F32, kind="ExternalOutput")

    with nc.allow_low_precision("fp32r matmul path"), tile.TileContext(nc) as tc:
        with (
            tc.tile_pool(name="xc", bufs=2) as xc_pool,
            tc.tile_pool(name="qk2", bufs=2) as qk2_pool,
            tc.tile_pool(name="wqk", bufs=2) as wqk_pool,
            tc.tile_pool(name="wconst", bufs=1) as wconst_pool,
            tc.tile_pool(name="vp", bufs=NTT) as vp_pool,
            tc.tile_pool(name="yt", bufs=NPAIR) as yt_pool,
            tc.tile_pool(name="small", bufs=1) as small_pool,
            tc.tile_pool(name="et", bufs=4) as et_pool,
            tc.tile_pool(name="ot", bufs=2) as ot_pool,
            tc.tile_pool(name="rsb", bufs=3) as rsb_pool,
            tc.tile_pool(name="recip", bufs=2) as recip_pool,
            tc.tile_pool(name="psmm", bufs=2, space="PSUM") as psmm_pool,
            tc.tile_pool(name="pso", bufs=2, space="PSUM") as pso_pool,
        ):
            pools = dict(
                xc=xc_pool, qk2=qk2_pool, wqk=wqk_pool, wconst=wconst_pool,
                vp=vp_pool, yt=yt_pool, small=small_pool, et=et_pool,
                ot=ot_pool, rsb=rsb_pool, recip=recip_pool,
                psmm=psmm_pool, pso=pso_pool,
            )
            drams = dict(
                xT=xT_d, wqk=wqk_d, wv=wv_d, wp=wp_d, mask=mask_d,
                ones=ones_d, bqk=bqk_d, bv=bv_d, out=out_d,
            )
            rep = tc.For_i(0, repeat, 1) if repeat > 1 else contextlib.nullcontext()
            with rep:
                _run_body(nc, has_bqk, has_bv, pools, drams)

    nc.compile()
    return nc


def _run_body(nc, has_bqk, has_bv, pools, drams):
    xT_r = drams["xT"][:].rearrange("(k p) t -> p k t", p=P)    # [P, NKT, T]
    wqk_r = drams["wqk"][:].rearrange("(k p) m -> p k m", p=P)  # [P, NKT, 1024]
    out_d = drams["out"]
    wv_r = drams["wv"][:].rearrange("(k p) n -> p k n", p=P)

    mask_sb = pools["small"].tile([P, 512], MMDT, name="mask_sb", tag="mask")
    ones_sb = pools["small"].tile([P, TQ], MMDT, name="ones_sb", tag="ones")

    bqk_sb = bv_sb = None
    if has_bqk:
        bqk_sb = pools["small"].tile([1, 4 * P * 2], MMDT, name="bqk_sb", tag="bqk")
        nc.sync.dma_start(bqk_sb[:], drams["bqk"][:])
    if has_bv:
        bv_sb = pools["small"].tile([1, DV], MMDT, name="bv_sb", tag="bv")
        nc.sync.dma_start(bv_sb[:], drams["bv"][:])

    wv_sb = pools["wconst"].tile([P, NKT, DV], MMDT, name="wv_sb", tag="w")

    vtiles = [None] * NTT
    ytiles = [
        pools["yt"].tile([P, T], MMDT, name=f"yt{a}", tag="yt")
        for a in range(NPAIR)
    ]

    wqk_tiles = {}
    qk2_tiles = {}

    def issue_pair_weights(p):
        wqk_h = pools["wqk"].tile([P, NKT, 2 * P], MMDT, name=f"wqkh{p}", tag="wqk")
        for k in range(NKT):
            # per-k slices so the first matmul can start after ~128KB
            nc.sync.dma_start(
                wqk_h[:, k, :], wqk_r[:, k, 2 * P * p:2 * P * (p + 1)]
            )
        wqk_tiles[p] = wqk_h
        qk2_tiles[p] = pools["qk2"].tile([P, 2, T], MMDT, name=f"qk2_{p}", tag="qk2")

    def produce_chunk(ps_list, t, with_v):
        """QK^T production for chunk t for each pair in ps_list (Q into psum
        cols 0:512, K into 512:1024 — separate banks — then one copy into the
        [P,2,T] layout)."""
        xct = pools["xc"].tile([P, NKT, TQ], MMDT, name=f"xct{ps_list[0]}_{t}", tag="xc")
        for k in range(NKT):
            nc.sync.dma_start(xct[:, k, :], xT_r[:, k, t * TQ:(t + 1) * TQ])
        if with_v and t == 0:
            nc.sync.dma_start(
                wv_sb[:], drams["wv"][:].rearrange("(k p) n -> p k n", p=P)
            )
            nc.sync.dma_start(mask_sb[:], drams["mask"][:])
            nc.sync.dma_start(ones_sb[:], drams["ones"][:])
        for p in ps_list:
            wqk_h = wqk_tiles[p]
            ps2 = pools["psmm"].tile([P, 2 * TQ], F32, name=f"psqk{p}_{t}", tag="mm2")
            for qk in range(2):
                for k in range(NKT):
                    nc.tensor.matmul(
                        ps2[:, qk * TQ:(qk + 1) * TQ],
                        wqk_h[:, k, qk * P:(qk + 1) * P],
                        xct[:, k, :],
                        start=(k == 0),
                        stop=(k == NKT - 1 and not has_bqk),
                    )
                if has_bqk:
                    nc.tensor.matmul(
                        ps2[:, qk * TQ:(qk + 1) * TQ],
                        bqk_sb[:, 2 * P * p + qk * P:2 * P * p + (qk + 1) * P],
                        ones_sb[0:1, :],
                        start=False, stop=True,
                    )
            nc.vector.tensor_copy(
                qk2_tiles[p][:, :, t * TQ:(t + 1) * TQ],
                ps2[:].rearrange("p (two n) -> p two n", n=TQ),
            )
        if with_v:
            # V' tiles [P tokens, NH, HD+1] (ones col appended); two token
            # tiles share one 2-bank psum tile
            for lp in range(2):
                psv = pools["psmm"].tile([P, 2 * TQ], F32, name=f"psv{t}_{lp}", tag="mm2")
                for half in range(2):
                    loc = 2 * lp + half
                    tt = t * (TQ // P) + loc
                    for k in range(NKT):
                        nc.tensor.matmul(
                            psv[:, half * DV:(half + 1) * DV],
                            xct[:, k, loc * P:(loc + 1) * P],
                            wv_sb[:, k, :],
                            start=(k == 0),
                            stop=(k == NKT - 1 and not has_bv),
                        )
                    if has_bv:
                        nc.tensor.matmul(
                            psv[:, half * DV:(half + 1) * DV],
                            ones_sb[0:1, 0:P], bv_sb[:],
                            start=False, stop=True,
                        )
                    vt = pools["vp"].tile([P, NH, HD + 1], MMDT, name=f"vt{tt}", tag="vt")
                    nc.vector.tensor_copy(
                        vt[:, :, 0:HD],
                        psv[:, half * DV:(half + 1) * DV].rearrange(
                            "p (h d) -> p h d", d=HD
                        ),
                    )
                    nc.vector.tensor_copy(
                        vt[:, :, HD:HD + 1],
                        ones_sb[:, 0:NH].rearrange("p (h o) -> p h o", o=1),
                    )
                    vtiles[tt] = vt

    def attention_group(pair, j, qgp):
        """One head (j) x one query-group pair (qgp): S^T, exp, mask, PV,
        then a fast-release copy and per-half normalization."""
        h = 2 * pair + j
        qk2 = qk2_tiles[pair]
        qT = qk2[64 * j:64 * j + HD, 0, :]
        kT = qk2[64 * j:64 * j + HD, 1, :]
        q0, q1 = 2 * qgp, 2 * qgp + 1
        nkb0 = (q0 + 1) * (TQ // P)
        nkb = (q1 + 1) * (TQ // P)
        ps_o = pools["pso"].tile([HD + 1, 2 * TQ], F32, name=f"pso{h}_{qgp}", tag="o2")
        for kb in range(nkb):
            in_q0 = kb < nkb0
            r0 = kb - q0 * (TQ // P)
            r1 = kb - q1 * (TQ // P)
            c0 = 128 * r0 if (in_q0 and r0 > 0) else 0
            c1 = 128 * r1 if r1 > 0 else 0
            ps2 = pools["psmm"].tile([P, 2 * TQ], F32, name=f"pss{h}_{qgp}_{kb}", tag="mm2")
            if in_q0:
                nc.tensor.matmul(
                    ps2[:, c0:TQ],
                    kT[:, kb * P:(kb + 1) * P],
                    qT[:, q0 * TQ + c0:(q0 + 1) * TQ],
                    start=True, stop=True,
                )
            nc.tensor.matmul(
                ps2[:, TQ + c1:2 * TQ],
                kT[:, kb * P:(kb + 1) * P],
                qT[:, q1 * TQ + c1:(q1 + 1) * TQ],
                start=True, stop=True,
            )
            et = pools["et"].tile([P, 2 * TQ], MMDT, name=f"et{h}_{qgp}_{kb}", tag="et")
            lo = c0 if in_q0 else TQ + c1
            # E^T = exp(S^T / sqrt(hd)) over the contiguous valid range
            nc.scalar.activation(
                et[:, lo:2 * TQ], ps2[:, lo:2 * TQ], EXP, scale=1.0 / np.sqrt(HD)
            )
            if in_q0 and r0 >= 0:
                # 128-wide diagonal band: zero the strictly-upper (k > q) part
                nc.vector.tensor_mul(
                    et[:, c0:c0 + P], et[:, c0:c0 + P], mask_sb[:, 384:384 + P]
                )
            if r1 >= 0:
                if c1 > 0:
                    # masked prefix: mask_sb[:, 0:384] is all zeros
                    nc.vector.tensor_copy(et[:, TQ:TQ + c1], mask_sb[:, 0:c1])
                nc.vector.tensor_mul(
                    et[:, TQ + c1:TQ + c1 + P],
                    et[:, TQ + c1:TQ + c1 + P],
                    mask_sb[:, 384:384 + P],
                )
            if in_q0:
                nc.tensor.matmul(
                    ps_o[:, c0:TQ],
                    vtiles[kb][:, h, :],
                    et[:, c0:TQ],
                    start=(kb == 0),
                    stop=(kb == nkb0 - 1),
                )
            nc.tensor.matmul(
                ps_o[:, TQ + c1:2 * TQ],
                vtiles[kb][:, h, :],
                et[:, TQ + c1:2 * TQ],
                start=(kb == 0),
                stop=(kb == nkb - 1),
            )
        # normalize each half: Y^T = O^T * broadcast(1/denom)
        for half, qi in ((0, q0), (1, q1)):
            sl = slice(half * TQ, (half + 1) * TQ)
            recip = pools["recip"].tile([1, TQ], MMDT, name=f"recip{h}_{qi}", tag="recip")
            nc.vector.reciprocal(recip[:], ps_o[HD:HD + 1, sl])
            r_sb = pools["rsb"].tile([HD, TQ], MMDT, name=f"rsb{h}_{qi}", tag="rsb")
            nc.gpsimd.partition_broadcast(r_sb[:], recip[0:1, :])
            yt_slice = ytiles[pair][64 * j:64 * j + HD, qi * TQ:(qi + 1) * TQ]
            nc.vector.tensor_mul(yt_slice, ps_o[0:HD, sl], r_sb[:])

    def proj_tt(tt, wp_sb):
        """c_proj partial rows tt*128..: out[t, c] = sum_a Y^T[a, t] Wp[a, c]."""
        ps2 = pools["psmm"].tile([P, 2 * TQ], F32, name=f"pspr{tt}", tag="mm2")
        for nn in range(C // TQ):
            for a in range(DV // P):
                nc.tensor.matmul(
                    ps2[:, nn * TQ:(nn + 1) * TQ],
                    ytiles[a][:, tt * P:(tt + 1) * P],
                    wp_sb[:, a, nn * TQ:(nn + 1) * TQ],
                    start=(a == 0),
                    stop=(a == DV // P - 1),
                )
        ot = pools["ot"].tile([P, 2 * TQ], F32, name=f"ot{tt}", tag="ot")
        nc.vector.tensor_copy(ot[:], ps2[:])
        nc.sync.dma_start(out_d[tt * P:(tt + 1) * P, :], ot[:])

    for half_pass in range(2):
        p0, p1 = 2 * half_pass, 2 * half_pass + 1
        issue_pair_weights(p0)
        issue_pair_weights(p1)
        for t in range(NCH):
            produce_chunk([p0, p1], t, with_v=(half_pass == 0))
        for pair in (p0, p1):
            for j in range(2):
                for qgp in range(NCH // 2):
                    attention_group(pair, j, qgp)

    wp_sb = pools["wconst"].tile([P, DV // P, C], MMDT, name="wp_sb", tag="w")
    nc.sync.dma_start(wp_sb[:], drams["wp"][:].rearrange("(a p) n -> p a n", p=P))
    for tt in range(NTT):
        proj_tt(tt, wp_sb)


_PROG_CACHE = {}


def _get_program(has_bqk: bool, has_bv: bool, repeat: int = 1):
    key = (has_bqk, has_bv, MM_DTYPE, repeat)
    if key not in _PROG_CACHE:
        _PROG_CACHE[key] = _build_program(has_bqk, has_bv, repeat)
    return _PROG_CACHE[key]


def make_mask() -> np.ndarray:
    # mask[i, c] = 1 iff i <= c - 384; cols 0:384 are all zero (used as a
    # zero-fill source); cols 384:512 are the 128-wide triangular band.
    i = np.arange(P)[:, None]
    c = np.arange(512)[None, :]
    return (i <= c - 384).astype(np.float32)


def shard_inputs(x, w_attn, b_attn, w_proj):
    """Build the 8 per-core input maps."""
    mask = make_mask()
    ones = np.ones((P, TQ), dtype=np.float32)
    has_bqk = bool(np.any(b_attn[:2 * C]))
    has_bv = bool(np.any(b_attn[2 * C:]))
    in_maps = []
    for core in range(N_CORES):
        b, half = divmod(core, 2)
        hs = half * 512  # col offset into the q/k/v regions
        # per-pair blocks of 256 cols: [Q_2i | Q_2i+1 | K_2i | K_2i+1]
        wq = w_attn[:, hs:hs + 512].reshape(C, NPAIR, 2 * HD)
        wk = w_attn[:, C + hs:C + hs + 512].reshape(C, NPAIR, 2 * HD)
        wqk = np.concatenate([wq, wk], axis=2).reshape(C, 4 * P * 2)
        m = {
            "xT": np.ascontiguousarray(np.asarray(x[b]).T),
            "wqk": np.ascontiguousarray(wqk),
            "wv": np.ascontiguousarray(w_attn[:, 2 * C + hs:2 * C + hs + 512]),
            "wp": np.ascontiguousarray(w_proj[hs:hs + 512, :]),
            "mask": mask,
            "ones": ones,
        }
        if has_bqk:
            bq = b_attn[hs:hs + 512].reshape(NPAIR, 2 * HD)
            bk = b_attn[C + hs:C + hs + 512].reshape(NPAIR, 2 * HD)
            m["bqk"] = np.ascontiguousarray(
                np.concatenate([bq, bk], axis=1).reshape(1, 4 * P * 2)
            ).astype(np.float32)
        if has_bv:
            m["bv"] = np.ascontiguousarray(
                b_attn[2 * C + hs:2 * C + hs + 512].reshape(1, DV)
            ).astype(np.float32)
        in_maps.append(m)
    return in_maps, has_bqk, has_bv


def kernel(x, w_attn, b_attn, w_proj, b_proj):
    x = np.asarray(x, dtype=np.float32)
    w_attn = np.asarray(w_attn, dtype=np.float32)
    b_attn = np.asarray(b_attn, dtype=np.float32)
    w_proj = np.asarray(w_proj, dtype=np.float32)
    b_proj = np.asarray(b_proj, dtype=np.float32)

    in_maps, has_bqk, has_bv = shard_inputs(x, w_attn, b_attn, w_proj)
    nc = _get_program(has_bqk, has_bv)
    res = run_bass_kernel_spmd(nc, in_maps, list(range(N_CORES))).results

    out = np.empty((B, T, C), dtype=np.float32)
    for b in range(B):
        out[b] = res[2 * b]["out"] + res[2 * b + 1]["out"] + b_proj
    return out
